# revision 3
# baseline (speedup 1.0000x reference)
"""Trainium2 Bass kernel for BasePropagationGraphPositionalEncoding, v3.

Computes, for each batch element b:
    out[b] = (sum_k coefs[k] * gr_kernel[b, k]) @ x[b] / sum_k coefs[k]
with coefs[k] = (1 - EPS)^k, EPS = 0.01, K = 9.

Sharding: batch dim B=8 across the 8 NeuronCores (data parallel).

v3 design (vs the f32 baseline):
  * gr_kernel slabs are cast f32 -> bf16 *during* the DMA (SWDGE/gpsimd
    path supports dtype conversion in flight). HBM read traffic is
    unchanged (the memory-bound term) but every SBUF byte downstream is
    halved and the DVE/PE work runs in bf16.
  * The weighted k-sum is restructured: DVE scalar_tensor_tensor has NO
    fast perf mode (1 elem/cycle always), so instead of a 9-op STT chain
    (~11 us/band ~= the DMA band cadence -> DVE was the co-bottleneck
    that throttled the stream), we do an exact Horner sum within 3
    k-groups (2 STT per group, 6 ops) producing H_g = G_{3g} +
    r*G_{3g+1} + r^2*G_{3g+2}, and fold the outer scales r^{3g} into the
    PE transpose step: per [128,128] chunk, 3 PSUM-accumulated
    transpose-matmuls against scaled identities c_g*I give
    wkT = sum_g c_g * H_g^T exactly. bf16 matmuls are 1-pass (4x faster
    than the fp32 LOW_HIGH path).
  * Rows are processed in groups of 256 (pairs, a=2) for fat 1 MB DMAs
    with 8 KB contiguous per-partition reads; the last two groups are
    single bands to shrink the end-of-stream tail.
  * Combine matmuls are emitted grouped by k-group (all A, then all B,
    then all C across chunks) so PE work on early-arriving slabs is not
    queued behind the stall waiting for the last slab.
"""

import sys

if "/opt/trn_rl_repo" not in sys.path:
    sys.path.insert(0, "/opt/trn_rl_repo")

import numpy as np

import concourse.bass as bass
import concourse.mybir as mybir
from concourse import tile
from concourse.bacc import Bacc
from concourse.masks import make_identity
from concourse.bass_utils import run_bass_kernel_spmd

# Problem shapes (hardcoded per the harness contract).
B, K, N, D = 8, 9, 1024, 64
EPS = 0.01
R = 1.0 - EPS
P = 128          # SBUF partitions
NT = N // P      # 8 column chunks of the [N, N] kernel

F32 = mybir.dt.float32
BF16 = mybir.dt.bfloat16

# Row groups: (start_row, A) with A rows per partition (group = A*128 rows).
# The last two groups are single bands to keep the end-of-stream tail
# short.
GROUPS = [(0, 2), (256, 2), (512, 2), (768, 1), (896, 1)]
NG = 3           # k-groups of 3 slabs each


def build_bass() -> bass.Bass:
    nc = Bacc()

    x_d = nc.dram_tensor("x_b", (N, D), F32, kind="ExternalInput")
    g_d = nc.dram_tensor("g_b", (K, N, N), F32, kind="ExternalInput")
    o_d = nc.dram_tensor("out_b", (N, D), F32, kind="ExternalOutput")

    coefs = R ** np.arange(K, dtype=np.float64)
    x_scale = float(1.0 / coefs.sum())
    # Identity diagonal scales for the 5 PSUM-combine terms:
    #   wk = H_A + r^3*H_B + r^6*G6 + r^7*G7 + r^8*G8
    # Slabs 6,7,8 feed the PE directly (no DVE op touches them), so the
    # last ~3.7 us of each group's stream goes straight into the combine
    # matmuls and the post-stream tail is just g8's stop-matmuls.
    c_outer = [1.0, float(R**3), float(R**6), float(R**7), float(R**8)]

    mult = mybir.AluOpType.mult
    add = mybir.AluOpType.add

    with tile.TileContext(nc) as tc:
        with (
            tc.tile_pool(name="consts", bufs=1) as consts,
            tc.tile_pool(name="gr", bufs=3) as gr_pool,
            tc.tile_pool(name="hp", bufs=2) as h_pool,
            tc.tile_pool(name="wkt", bufs=2) as wkt_pool,
            tc.tile_pool(name="outp", bufs=2) as out_pool,
            tc.tile_pool(name="ps_t", bufs=1, space=bass.MemorySpace.PSUM) as ps_tr,
            tc.tile_pool(name="ps_e", bufs=2, space=bass.MemorySpace.PSUM) as ps_emb,
        ):
            # --- DMA issue: one 1 MB (A=2) or 512 KB (A=1) cast-DMA per
            # slab k.  All loads ride the single SWDGE queue (gpsimd is
            # the only engine that can cast f32->bf16 in flight), so
            # keep gpsimd's instruction queue free of anything else
            # until the steady-state prefetch is rolling.
            def issue_group(j):
                Rr, A = GROUPS[j]
                tiles = [None] * K
                # Natural arrival order: the last DVE input (g5) lands
                # ~3.7 us before the group's final byte; g6..g8 are
                # DVE-free PE terms.
                for k in range(K):
                    t = gr_pool.tile([P, 2, N], BF16, tag=f"g{k}", name=f"g{j}_{k}")
                    if A == 2:
                        # Two 2D DMAs (one per a): ~6% faster sustained
                        # than a single 3D DMA of the same bytes.
                        src = g_d[k, Rr : Rr + 2 * P, :].rearrange(
                            "(p a) m -> p a m", p=P
                        )
                        for a in range(2):
                            nc.gpsimd.dma_start(t[:, a, :], src[:, a, :])
                    else:
                        nc.gpsimd.dma_start(t[:, 0, :], g_d[k, Rr : Rr + P, :])
                    tiles[k] = t
                return tiles

            pending = {}
            pending[0] = issue_group(0)

            # x load on sync (HWDGE; needed only at ~25 us).
            x_f32 = consts.tile([P, NT, D], F32)
            nc.sync.dma_start(x_f32[:], x_d.rearrange("(c p) d -> p c d", p=P))

            # Identity on gpsimd after group 0's doorbells are rung.
            ident_raw = consts.tile([P, P], F32)
            make_identity(nc, ident_raw)

            pending[1] = issue_group(1)

            # Scaled bf16 identities (single-engine DVE deps for the PE).
            idents = []
            for gi in range(len(c_outer)):
                idg = consts.tile([P, P], BF16, name=f"ident{gi}")
                nc.vector.tensor_scalar_mul(idg[:], ident_raw[:], c_outer[gi])
                idents.append(idg)

            # x scaled by 1/sum(coefs) and cast to bf16 on DVE.
            x_sb = consts.tile([P, NT, D], BF16)
            nc.vector.tensor_scalar_mul(x_sb[:], x_f32[:], x_scale)

            for j, (Rr, A) in enumerate(GROUPS):
                if j + 2 < len(GROUPS):
                    pending[j + 2] = issue_group(j + 2)
                g_ts = pending.pop(j)

                # Exact Horner k-sums on DVE:
                #   H_A = (G1*r + G0) + G2*r^2
                #   H_B = (G4*r + G3) + G5*r^2
                # G6, G7, G8 never touch DVE (direct PE terms).
                Hs = [None] * 2
                def new_h(gi):
                    Hg = h_pool.tile([P, 2, N], BF16, tag=f"H{gi}", name=f"H_{gi}")
                    Hs[gi] = Hg
                    return Hg
                def stt(out, in0, s, in1):
                    nc.vector.scalar_tensor_tensor(
                        out[:, 0:A, :], in0[:, 0:A, :], s,
                        in1[:, 0:A, :], op0=mult, op1=add,
                    )
                HA = new_h(0)
                stt(HA, g_ts[1], R, g_ts[0])
                stt(HA, g_ts[2], R * R, HA)
                HB = new_h(1)
                stt(HB, g_ts[4], R, g_ts[3])
                stt(HB, g_ts[5], R * R, HB)

                # PE combine: wkT chunk (a, c) = sum_g c_g * H_g[:, a, c]^T
                # via PSUM-accumulated matmuls against scaled identities.
                # PSUM accumulation is BANK-granular: 'start' clears the
                # whole 2 KB bank, so each [P, 4, P] bank tile is ONE
                # accumulation group (12 matmuls: 4 chunks x 3 k-groups,
                # start on the first, stop on the last).  Emitted g-major
                # so A/B work runs while C's slabs are still streaming.
                # Term order matches data-readiness; g8 (stop) is the
                # only term gated on the final slab of the group.
                terms = [
                    (Hs[0], 0), (Hs[1], 1),
                    (g_ts[6], 2), (g_ts[7], 3), (g_ts[8], 4),
                ]
                psums = {}
                for ti, (lhs_tile, gi) in enumerate(terms):
                    for a in range(A):
                        for c in range(NT):
                            h = c // 4
                            if ti == 0 and c % 4 == 0:
                                psums[(a, h)] = ps_tr.tile(
                                    [P, 4, P], F32, tag=f"pt{a}{h}",
                                    name=f"pt{a}_{h}",
                                )
                            nc.tensor.matmul(
                                psums[(a, h)][:, c % 4, :],
                                lhs_tile[:, a, c * P : (c + 1) * P],
                                idents[gi][:],
                                start=(ti == 0 and c % 4 == 0),
                                stop=(ti == len(terms) - 1 and c % 4 == 3),
                            )

                wkts = {}
                for a in range(A):
                    for h in range(2):
                        w = wkt_pool.tile(
                            [P, 4, P], BF16, tag=f"wkt{a}{h}", name=f"wkt{a}_{h}"
                        )
                        nc.scalar.copy(w[:], psums[(a, h)][:])
                        wkts[(a, h)] = w

                # emb rows {n = Rr + A*j + a}: PSUM-accumulate over the 8
                # contraction chunks against x.
                o_t = out_pool.tile([P, 2 * D], F32, tag="o")
                for a in range(A):
                    emb = ps_emb.tile([P, D], F32, tag=f"pe{a}")
                    for c in range(NT):
                        nc.tensor.matmul(
                            emb[:],
                            wkts[(a, c // 4)][:, c % 4, :],
                            x_sb[:, c, :],
                            start=(c == 0),
                            stop=(c == NT - 1),
                        )
                    nc.scalar.copy(o_t[:, a * D : (a + 1) * D], emb[:])

                if A == 2:
                    dst = o_d[Rr : Rr + 2 * P, :].rearrange(
                        "(p a) d -> p (a d)", p=P
                    )
                    nc.sync.dma_start(dst, o_t[:, 0 : 2 * D])
                else:
                    nc.sync.dma_start(o_d[Rr : Rr + P, :], o_t[:, 0:D])

    nc.compile()
    return nc


_NC = None


def _get_nc() -> bass.Bass:
    global _NC
    if _NC is None:
        _NC = build_bass()
    return _NC


def run(x: np.ndarray, gr_kernel: np.ndarray, **spmd_kwargs):
    """Run the SPMD kernel on cores 0-7; returns BassKernelResults."""
    nc = _get_nc()
    in_maps = [
        {
            "x_b": np.ascontiguousarray(x[b], dtype=np.float32),
            "g_b": np.ascontiguousarray(gr_kernel[b], dtype=np.float32),
        }
        for b in range(B)
    ]
    return run_bass_kernel_spmd(nc, in_maps, core_ids=list(range(B)), **spmd_kwargs)


def kernel(x: np.ndarray, gr_kernel: np.ndarray) -> np.ndarray:
    res = run(np.asarray(x), np.asarray(gr_kernel))
    out = np.stack([res.results[b]["out_b"] for b in range(B)], axis=0)
    return out.astype(np.float32, copy=False)


if __name__ == "__main__":
    rng = np.random.default_rng(0)
    x = rng.standard_normal((B, N, D), dtype=np.float32)
    g = rng.standard_normal((B, K, N, N), dtype=np.float32)
    out = kernel(x, g)
    coefs = (1.0 - EPS) ** np.arange(K)
    wk = np.einsum("k,bknm->bnm", coefs, g)
    ref = np.matmul(wk, x) / coefs.sum()
    err = np.linalg.norm(out - ref) / np.linalg.norm(ref)
    print("self-check rel err:", err)
